# revision 1
# baseline (speedup 1.0000x reference)
"""Trainium2 Bass kernel for the ConstraintFuser GNN message-passing module.

Computation (per batch row b, C=50 constraints, D=512):
    h = entity_emb[heads[b]]            # [C, D] gather
    t = entity_emb[tails[b]]            # [C, D] gather
    r = rel_emb[rels[b]]                # [C, D] gather
    score[c]  = <q[b], h[c]>            # [C]
    pooled    = sum_c score[c] * (t[c] + r[c])
    out[b]    = relu(pooled @ w1 + b1) @ w2 + b2 + q[b]

Sharding: data-parallel over the batch dim across 8 NeuronCores (256 rows
each); embedding tables + FFN weights replicated to every core.

Per-core kernel layout: batch tiles of 128 rows on the SBUF partition dim.
For each constraint c, gather h/t/r rows for all 128 batch rows (one
indirect DMA per group of G constraints), compute scores with a fused
multiply+reduce on DVE, then accumulate score-weighted t/r rows into PSUM
with diag(score) matmuls on the tensor engine.
"""

import os
import sys

sys.path.insert(0, "/opt/trn_rl_repo")

import numpy as np
from contextlib import ExitStack

from concourse import bacc, bass, mybir, tile
from concourse.bass import IndirectOffsetOnAxis
from concourse.bass_utils import run_bass_kernel_spmd
from concourse.masks import make_identity

P = 128          # SBUF partitions / batch-tile size
D = 512          # embedding dim
C = 50           # constraints per batch row
H = 51           # FFN hidden dim
NE = 100001      # entity table rows
NR = 501         # relation table rows
N_CORES = 8
B = 2048
BL = B // N_CORES        # 256 batch rows per core
NT = BL // P             # 2 batch tiles per core
# NOTE: HW indirect DMA supports exactly ONE gathered row per partition per
# instruction (multi-index-per-partition gathers return garbage on HW even
# though CoreSim accepts them) -> one [128, D] gather per constraint.

F32 = mybir.dt.float32
I32 = mybir.dt.int32
F32R = mybir.dt.float32r

# Pooled-matmul dtype knob: float32 (safe, 4 cyc/row) or float32r (1 cyc/row).
# fp32r operands must be *produced* as fp32r (BIR verifier rule), so the t/r
# gather tiles are written as f32r by the SWDGE cast and diag by the ACT copy.
MM_DT = os.environ.get("KERNEL_MM_DT", "float32r")

# Number of SWDGE queues to spread indirect gathers over (1..4). One queue
# serializes the 300 gather instructions (~0.6us bubble each from descriptor
# generation + completion-receipt); multiple rings overlap those phases.
N_SWDGE_Q = int(os.environ.get("KERNEL_SWDGE_Q", "4"))


def _gather(nc, out_ap, table_ap, idx_ap, qi):
    inst = nc.gpsimd.indirect_dma_start(
        out=out_ap,
        out_offset=None,
        in_=table_ap,
        in_offset=IndirectOffsetOnAxis(ap=idx_ap, axis=0),
    )
    q = qi % N_SWDGE_Q
    if q:
        inst.ins.queue = f"qPoolDynamic{q}"
    return inst


def build_nc():
    nc = bacc.Bacc("TRN2", target_bir_lowering=False, debug=False, num_swdge_queues=N_SWDGE_Q)

    q_d = nc.dram_tensor("query_embedding", [BL, D], F32, kind="ExternalInput")
    heads_d = nc.dram_tensor("heads", [BL, C], I32, kind="ExternalInput")
    tails_d = nc.dram_tensor("tails", [BL, C], I32, kind="ExternalInput")
    rels_d = nc.dram_tensor("rels", [BL, C], I32, kind="ExternalInput")
    ent_d = nc.dram_tensor("entity_emb", [NE, D], F32, kind="ExternalInput")
    rel_d = nc.dram_tensor("rel_emb", [NR, D], F32, kind="ExternalInput")
    w1_d = nc.dram_tensor("w1", [D, H], F32, kind="ExternalInput")
    b1_d = nc.dram_tensor("b1", [H], F32, kind="ExternalInput")
    w2_d = nc.dram_tensor("w2", [H, D], F32, kind="ExternalInput")
    b2_d = nc.dram_tensor("b2", [D], F32, kind="ExternalInput")
    out_d = nc.dram_tensor("out", [BL, D], F32, kind="ExternalOutput")

    RDT = F32R if MM_DT == "float32r" else F32

    with tile.TileContext(nc) as tc, ExitStack() as ctx:
        constp = ctx.enter_context(tc.tile_pool(name="const", bufs=1))
        iop = ctx.enter_context(tc.tile_pool(name="io", bufs=2))
        gp = ctx.enter_context(tc.tile_pool(name="gather", bufs=8))
        dgp = ctx.enter_context(tc.tile_pool(name="diag", bufs=4))
        scp = ctx.enter_context(tc.tile_pool(name="scratch", bufs=2))
        psp = ctx.enter_context(tc.tile_pool(name="ps_pool", bufs=2, space="PSUM"))
        pst = ctx.enter_context(tc.tile_pool(name="ps_tr", bufs=2, space="PSUM"))
        psm = ctx.enter_context(tc.tile_pool(name="ps_mid", bufs=1, space="PSUM"))
        pso = ctx.enter_context(tc.tile_pool(name="ps_out", bufs=2, space="PSUM"))

        identity = constp.tile([P, P], F32)
        make_identity(nc, identity[:])

        # w1 [512, 51] -> SBUF [128, 4, 51]; chunk f holds rows f*128..f*128+127
        w1_t = constp.tile([P, 4, H], F32)
        nc.sync.dma_start(out=w1_t[:], in_=w1_d.ap().rearrange("(f p) h -> p f h", p=P))
        # w2 [51, 512] + b2 appended as row 51 (ones-row trick folds the bias in)
        w2b = constp.tile([H + 1, D], F32)
        nc.sync.dma_start(out=w2b[:H, :], in_=w2_d.ap())
        nc.sync.dma_start(out=w2b[H : H + 1, :], in_=b2_d.ap()[None, :])
        b1_t = constp.tile([H, 1], F32)
        nc.sync.dma_start(out=b1_t[:], in_=b1_d.ap()[:, None])

        for ti in range(NT):
            r0 = ti * P
            q_t = iop.tile([P, D], F32)
            nc.sync.dma_start(out=q_t[:], in_=q_d.ap()[r0 : r0 + P, :])
            heads_t = iop.tile([P, C], I32)
            nc.sync.dma_start(out=heads_t[:], in_=heads_d.ap()[r0 : r0 + P, :])
            tails_t = iop.tile([P, C], I32)
            nc.sync.dma_start(out=tails_t[:], in_=tails_d.ap()[r0 : r0 + P, :])
            rels_t = iop.tile([P, C], I32)
            nc.sync.dma_start(out=rels_t[:], in_=rels_d.ap()[r0 : r0 + P, :])

            S = iop.tile([P, C], F32)
            pooled_ps = psp.tile([P, D], F32, space="PSUM")

            n_mm = 2 * C
            mi = 0
            for c in range(C):
                h_t = gp.tile([P, D], F32)
                _gather(nc, h_t[:], ent_d.ap(), heads_t[:, c : c + 1], 3 * c)
                t_t = gp.tile([P, D], RDT)
                _gather(nc, t_t[:], ent_d.ap(), tails_t[:, c : c + 1], 3 * c + 1)
                r_t = gp.tile([P, D], RDT)
                _gather(nc, r_t[:], rel_d.ap(), rels_t[:, c : c + 1], 3 * c + 2)
                # score[:, c] = sum_d q * h_c
                tout = scp.tile([P, D], F32)
                nc.vector.tensor_tensor(
                    out=tout[:], in0=q_t[:], in1=h_t[:], op=mybir.AluOpType.mult
                )
                nc.vector.tensor_reduce(
                    out=S[:, c : c + 1],
                    in_=tout[:],
                    axis=mybir.AxisListType.X,
                    op=mybir.AluOpType.add,
                )
                # diag(score_c) on the scalar engine
                diag = dgp.tile([P, P], RDT)
                nc.scalar.activation(
                    out=diag[:],
                    in_=identity[:],
                    func=mybir.ActivationFunctionType.Copy,
                    scale=S[:, c : c + 1],
                )
                # pooled += diag(score_c) @ t_c ; pooled += diag(score_c) @ r_c
                nc.tensor.matmul(
                    out=pooled_ps[:],
                    lhsT=diag[:],
                    rhs=t_t[:],
                    start=(mi == 0),
                    stop=(mi == n_mm - 1),
                )
                mi += 1
                nc.tensor.matmul(
                    out=pooled_ps[:],
                    lhsT=diag[:],
                    rhs=r_t[:],
                    start=False,
                    stop=(mi == n_mm - 1),
                )
                mi += 1

            # ---- FFN + residual ----
            pooled_sb = iop.tile([P, D], F32)
            nc.scalar.copy(out=pooled_sb[:], in_=pooled_ps[:])
            # transpose pooled -> pT [128 d-chunk, 4, 128 b]
            pT = iop.tile([P, 4, P], F32)
            for f in range(4):
                tps = pst.tile([P, P], F32, space="PSUM")
                nc.tensor.transpose(
                    out=tps[:], in_=pooled_sb[:, f * P : (f + 1) * P], identity=identity[:]
                )
                nc.scalar.copy(out=pT[:, f, :], in_=tps[:])
            # mid^T [51, 128] = sum_f w1_f^T @ pT_f
            mid_ps = psm.tile([H, P], F32, space="PSUM")
            for f in range(4):
                nc.tensor.matmul(
                    out=mid_ps[:],
                    lhsT=w1_t[:, f, :],
                    rhs=pT[:, f, :],
                    start=(f == 0),
                    stop=(f == 3),
                )
            mid_sb = iop.tile([H + 1, P], F32)
            nc.vector.memset(mid_sb[:, :], 1.0)
            nc.scalar.activation(
                out=mid_sb[:H, :],
                in_=mid_ps[:],
                func=mybir.ActivationFunctionType.Relu,
                bias=b1_t[:],
                scale=1.0,
            )
            # out2 [128 b, 512] = mid^T.T @ [w2; b2]
            out2_ps = pso.tile([P, D], F32, space="PSUM")
            nc.tensor.matmul(
                out=out2_ps[:], lhsT=mid_sb[:], rhs=w2b[:], start=True, stop=True
            )
            out_sb = iop.tile([P, D], F32)
            nc.vector.tensor_tensor(
                out=out_sb[:], in0=out2_ps[:], in1=q_t[:], op=mybir.AluOpType.add
            )
            nc.sync.dma_start(out=out_d.ap()[r0 : r0 + P, :], in_=out_sb[:])

    nc.compile()
    return nc


_NC_CACHE = None


def _get_nc():
    global _NC_CACHE
    if _NC_CACHE is None:
        _NC_CACHE = build_nc()
    return _NC_CACHE


def _in_maps(inputs):
    maps = []
    for i in range(N_CORES):
        sl = slice(i * BL, (i + 1) * BL)
        maps.append(
            {
                "query_embedding": np.ascontiguousarray(
                    np.asarray(inputs["query_embedding"], dtype=np.float32)[sl]
                ),
                "heads": np.ascontiguousarray(np.asarray(inputs["heads"], dtype=np.int32)[sl]),
                "tails": np.ascontiguousarray(np.asarray(inputs["tails"], dtype=np.int32)[sl]),
                "rels": np.ascontiguousarray(np.asarray(inputs["rels"], dtype=np.int32)[sl]),
                "entity_emb": np.asarray(inputs["entity_emb"], dtype=np.float32),
                "rel_emb": np.asarray(inputs["rel_emb"], dtype=np.float32),
                "w1": np.asarray(inputs["w1"], dtype=np.float32),
                "b1": np.asarray(inputs["b1"], dtype=np.float32),
                "w2": np.asarray(inputs["w2"], dtype=np.float32),
                "b2": np.asarray(inputs["b2"], dtype=np.float32),
            }
        )
    return maps


def kernel(**inputs) -> np.ndarray:
    nc = _get_nc()
    res = run_bass_kernel_spmd(nc, _in_maps(inputs), core_ids=list(range(N_CORES)))
    out = np.concatenate([res.results[i]["out"] for i in range(N_CORES)], axis=0)
    return np.asarray(out, dtype=np.float32)


def run_traced(inputs):
    """Dev helper: run on HW with NTFF tracing; returns BassKernelResults."""
    nc = _get_nc()
    return run_bass_kernel_spmd(
        nc, _in_maps(inputs), core_ids=list(range(N_CORES)), trace=True
    )

